# revision 11
# baseline (speedup 1.0000x reference)
"""HardTripletLoss Trainium2 kernel.

Reference computation (B=256, C=1000, D=300):
  relations[b,c] = ||emb[b*C+c] - att[b*C+c] + 1e-6||_2          [B, C]
  hardest_positive[c] = max_b relations[b,c] * onehot(labels)[b,c]
  mx[c]              = max_b relations[b,c]
  hardest_negative[c] = min_b (relations[b,c] + mx[c]*onehot[b,c])
  loss = sum(relu(hp - hn + 1)) / (count(relu(...) > 1e-16) + 1e-16)

Sharding: data-parallel over B across 8 cores (32 b's per core, each a
contiguous 32000-row chunk of the (B*C, D) tensors). Each core computes
squared distances rel_sq[P, R, BL] (class c = R*p + r) and 4 per-class
partial reductions [C] over local b:
  cmax = max_b rel_sq            pmax = max_b over positives of rel_sq
  umin = min_b over negatives    mmin = min_b over positives
(masking is exact: +/-1e30 select-style masks via min/max ALU ops, no
additive-cancellation error). Host all-reduces the [4, C] partials over
cores, takes sqrt (monotone, commutes with max/min), and finishes the
tiny [C]-sized tail: hn = min(umin, cmax_r + mmin), loss scalar.

DMA strategy (memory-bound: 76.8 MB/core of f32 input). Measured HW
facts driving the design:
  - the binding resource mid-stream is per-SDMA-engine throughput
    (~25.5 of ~29 GB/s read-side x 16 engines ~= 404 GB/s). The loads
    CAST f32->f16 in flight (SWDGE CCE) so the SBUF write side never
    binds. (f16 rounding is ~4e-6 relative on the loss; gate is 2e-2.)
  - all 64 chunk loads are spread over the 4 SWDGE rings (4 fixed DMA
    engines each) -- plain gpsimd dma_start is pinned to ring 0 by
    bass, so the InstDMACopy queue name is retargeted post-hoc to
    qPoolDynamic{1,2,3}. The 2 HWDGE queues share engines 0-4 and
    cannot cast; they carry only the tiny mask/out transfers.
  - partition p holds R=8 consecutive rows (c = 8p + r): each per-b load
    is a dense 1.2 MB 2D DMA, 9.6 KB contiguous HBM lines, 125 descs.
  - DMA-instruction issue on the in-order Pool engine costs ~0.7us
    each, so fewer+bigger DMAs win (half-splitting every chunk was a
    33% regression). The last chunk pair instead gets DEDICATED tiles:
    those 4 whole-chunk loads issue with no tile-free wait, killing an
    ~8us end-of-stream issue stall. io bufs=7 for the streamed chunks
    (6 and 8 are both measurably worse).
Compute: DVE subtracts e-a (f16); ACT Square+accum_out makes the row
sums for 6 of 8 rows (PSUM junk output keeps its writes off SBUF), DVE
mult+reduce the other 2 (4/4 near the end so tail buffers free sooner).
rel_sq is stored r-major [P, R, BL] so the masked per-class reductions
are 7 whole-tile DVE ops (a per-(r,b-range) formulation was 56 tiny ops
= ~12us of pure tail at ~205ns/op). One wave covers b<BL-NT mid-stream;
the last NT raw rel columns are copied (idle GpSimd) next to the wave
partials in one contiguous [P, 4+NT, R] tile and ship to the host in a
single dense output DMA; the host folds those NT b's into the
all-reduce in numpy -- so after the last DMA byte the device only runs
sub + squares + copy + a 16 KB store (~8us tail instead of ~22us).
"""

import numpy as np

B, C, D = 256, 1000, 300
M = 8            # cores
BL = B // M      # 32 local anchors per core
P = 125          # partitions; partition p holds classes c = R*p + r
R = C // P       # 8 consecutive rows per partition
NT = 4           # trailing b's whose raw rel columns go to the host
BIG = 1.0e30
EPS_PD = 1e-6
MARGIN = 1.0

_STATE = {}


def _build():
    import concourse.tile as tile
    from concourse import bacc, mybir

    nc = bacc.Bacc("TRN2", target_bir_lowering=False, debug=False,
                   num_devices=M, num_swdge_queues=4)
    dt = mybir.dt.float32
    emb = nc.dram_tensor("emb", [BL * C, D], dt, kind="ExternalInput").ap()
    att = nc.dram_tensor("att", [BL * C, D], dt, kind="ExternalInput").ap()
    msk = nc.dram_tensor("msk", [P, R, BL], dt, kind="ExternalInput").ap()
    out = nc.dram_tensor("out", [P, 4 + NT, R], dt, kind="ExternalOutput").ap()

    emb_v = emb.rearrange("(b p r) d -> b p r d", b=BL, p=P, r=R)
    att_v = att.rearrange("(b p r) d -> b p r d", b=BL, p=P, r=R)
    f16 = mybir.dt.float16

    Alu = mybir.AluOpType
    Act = mybir.ActivationFunctionType
    Ax = mybir.AxisListType

    def swdge(dst, src, q):
        """gpsimd dma_start retargeted to SWDGE ring q (0-3)."""
        inst = nc.gpsimd.dma_start(dst, src)
        if q:
            inst.ins.queue = f"qPoolDynamic{q}"
        return inst

    with tile.TileContext(nc) as tc:
        with (
            tc.tile_pool(name="io", bufs=7) as io_pool,
            tc.tile_pool(name="dif", bufs=2) as dif_pool,
            tc.tile_pool(name="small", bufs=1) as small_pool,
            tc.psum_pool(name="ps", bufs=2) as ps_pool,
        ):
            mask_t = small_pool.tile([P, R, BL], dt, tag="mask")
            nc.sync.dma_start(mask_t[:], msk[:])
            mask2_t = small_pool.tile([P, R, BL], dt, tag="mask2")
            nc.vector.tensor_scalar_mul(mask2_t[:], mask_t[:], -1.0)
            # rel_t[:, r, b] holds rel_sq of (b, c=R*p+r) -- r-major so the
            # reduce wave is whole-tile ops
            rel_t = small_pool.tile([P, R, BL], dt, tag="rel")
            # ACT's mandatory main output goes to PSUM: junk f32 writes to
            # SBUF (1.2 MB/chunk) were stealing SBUF write bandwidth from DMA
            junk_t = ps_pool.tile([P, D], dt, tag="junk", bufs=1)
            sq_t = ps_pool.tile([P, 4, D], dt, tag="sq", bufs=1)
            waste_t = small_pool.tile([P, R, BL - NT], dt, tag="waste")
            # rows 0-3: wave partials; rows 4..4+NT: raw rel tail columns.
            # One contiguous tile -> ONE dense 224 B/partition output DMA (a
            # strided out2 source was 1000x 12 B descriptors = ~4.5 us).
            part_t = small_pool.tile([P, 4 + NT, R], dt, tag="part")

            def reduce_wave(part, b0, b1):
                """Masked per-class partial min/max over b in [b0, b1).

                7 whole-tile DVE ops (a per-(r,b-range) formulation was 56
                tiny ops = ~12 us of pure tail at ~205ns/op). The scheduler
                statically interleaves them into the trailing chunks' DVE
                windows; with NT=4 trailing chunks they all land before the
                chunk-31 -> copy -> out chain, off the critical path.
                Masking via min/max with +-1e30 select masks is exact.
                (GpSimd can't take them: its tensor_reduce is axis=C only.)
                """
                n = b1 - b0
                rel_v = rel_t[:, :, b0:b1]
                m_v = mask_t[:, :, b0:b1]
                m2_v = mask2_t[:, :, b0:b1]
                w = waste_t[:, :, :n]
                nc.vector.tensor_reduce(part[:, 0, :], rel_v,
                                        axis=Ax.X, op=Alu.max)
                nc.vector.tensor_tensor(w, rel_v, m_v, op=Alu.min)
                nc.vector.tensor_reduce(part[:, 1, :], w,
                                        axis=Ax.X, op=Alu.max)
                nc.vector.tensor_tensor(w, rel_v, m_v, op=Alu.max)
                nc.vector.tensor_reduce(part[:, 2, :], w,
                                        axis=Ax.X, op=Alu.min)
                nc.vector.tensor_tensor(w, rel_v, m2_v, op=Alu.max)
                nc.vector.tensor_reduce(part[:, 3, :], w,
                                        axis=Ax.X, op=Alu.min)

            # f32 -> f16 cast during the DMA (SWDGE CCE): halves the
            # SBUF-write bytes; the f16 rounding is far under the 2e-2 gate.
            for b in range(BL):
                # whole contiguous 1.2 MB b-chunk, 9.6 KB partition lines.
                # The last chunk pair gets dedicated buffers: those loads
                # issue with no tile-free wait (the in-order issue engine
                # stalled ~8us there otherwise), and the 4 full loads land
                # one per ring so the final drain runs ring-parallel.
                if b < BL - 2:
                    e_t = io_pool.tile([P, R, D], f16, tag="e")
                    a_t = io_pool.tile([P, R, D], f16, tag="a")
                else:
                    e_t = io_pool.tile([P, R, D], f16, tag=f"e_tl{b & 1}",
                                       bufs=1)
                    a_t = io_pool.tile([P, R, D], f16, tag=f"a_tl{b & 1}",
                                       bufs=1)
                swdge(e_t[:], emb_v[b], (2 * b) % 4)
                swdge(a_t[:], att_v[b], (2 * b + 1) % 4)
                d_t = dif_pool.tile([P, R, D], f16, tag="d")
                nc.vector.tensor_sub(d_t[:], e_t[:], a_t[:])
                # split the square+rowsum rows between ACT and DVE; even the
                # split (4/4) for the last 10 chunks so tail buffers free
                # sooner and the final loads issue without stalling
                n_dve = 2 if b < BL - 10 else 4
                for r in range(R - n_dve):
                    # accum_out gives the 300-wide row-sum for free (f32)
                    nc.scalar.activation(
                        junk_t[:], d_t[:, r, :], Act.Square,
                        bias=0.0, scale=1.0,
                        accum_out=rel_t[:, r, b:b + 1],
                    )
                nc.vector.tensor_tensor(
                    sq_t[:, :n_dve], d_t[:, R - n_dve:, :],
                    d_t[:, R - n_dve:, :], op=Alu.mult)
                nc.vector.tensor_reduce(
                    rel_t[:, R - n_dve:, b:b + 1], sq_t[:, :n_dve],
                    axis=Ax.X, op=Alu.add)
                if b == BL - NT - 1:
                    # one wave over b < BL-NT, overlapped with the last NT
                    # chunks' DMA + compute
                    reduce_wave(part_t, 0, BL - NT)

            # the last NT raw rel columns go to the host verbatim; the host
            # folds those b's into the all-reduce directly (numpy, [C]-sized).
            # tensor_copy on the otherwise-idle GpSimd keeps DVE free.
            for j in range(NT):
                nc.gpsimd.tensor_copy(part_t[:, 4 + j, :],
                                      rel_t[:, :, BL - NT + j])
            nc.sync.dma_start(out[:], part_t[:])
    nc.compile()
    return nc


def _get_nc():
    if "nc" not in _STATE:
        _STATE["nc"] = _build()
    return _STATE["nc"]


def _make_masks(labels_np):
    """Per-core select masks msk[p, r, b] = +BIG if labels[b]==R*p+r else -BIG."""
    masks = []
    c_of_pr = R * np.arange(P)[:, None] + np.arange(R)[None, :]     # [P, R]
    for m in range(M):
        lb = labels_np[m * BL:(m + 1) * BL].astype(np.int64)        # [BL]
        match = c_of_pr[:, :, None] == lb[None, None, :]            # [P, R, BL]
        mask = np.where(match, np.float32(BIG), np.float32(-BIG))
        masks.append(np.ascontiguousarray(mask, dtype=np.float32))
    return masks


def _run_device(attributes, embeddings, labels_np, trace=False):
    from concourse.bass_utils import run_bass_kernel_spmd
    nc = _get_nc()
    masks = _make_masks(labels_np)
    in_maps = []
    for m in range(M):
        sl = slice(m * BL * C, (m + 1) * BL * C)
        in_maps.append({
            "emb": embeddings[sl],
            "att": attributes[sl],
            "msk": masks[m],
        })
    return run_bass_kernel_spmd(nc, in_maps, list(range(M)), trace=trace)


def _combine(results, labels_np):
    """All-reduce the per-core partials (plus NT raw tail columns) and
    finish the loss on host."""
    cmax = np.full(C, -np.inf)
    pmax = np.full(C, -np.inf)
    umin = np.full(C, np.inf)
    mmin = np.full(C, np.inf)
    for m in range(M):
        # device out [P, 4+NT, R] (class c = R*p + r) -> [4+NT, C]
        pk = np.transpose(results[m]["out"].astype(np.float64),
                          (1, 0, 2)).reshape(4 + NT, C)
        cmax = np.maximum(cmax, pk[0])
        pmax = np.maximum(pmax, pk[1])
        umin = np.minimum(umin, pk[2])
        mmin = np.minimum(mmin, pk[3])
        # raw rel_sq for the NT trailing b's
        tail = pk[4:]
        for j in range(NT):
            lb = int(labels_np[m * BL + BL - NT + j])
            row = tail[j]
            cmax = np.maximum(cmax, row)
            neg = row.copy()
            neg[lb] = np.inf
            umin = np.minimum(umin, neg)
            pmax[lb] = max(pmax[lb], row[lb])
            mmin[lb] = min(mmin[lb], row[lb])
    # squared space -> distances (max/min commute with sqrt on [0, inf))
    mx = np.sqrt(np.maximum(cmax, 0.0))
    hp = np.sqrt(np.clip(pmax, 0.0, None))    # -BIG (no positive) -> 0
    umin_r = np.sqrt(np.clip(umin, 0.0, None))  # +BIG sentinel stays huge
    mmin_r = np.sqrt(np.clip(mmin, 0.0, None))
    hn = np.minimum(umin_r, mx + mmin_r)
    triplet = np.maximum(hp - hn + MARGIN, 0.0)
    num_hard = np.sum(triplet > 1e-16)
    loss = np.sum(triplet) / (num_hard + 1e-16)
    return np.float32(loss)


def kernel(attributes, embeddings, labels):
    attributes = np.ascontiguousarray(np.asarray(attributes, dtype=np.float32))
    embeddings = np.ascontiguousarray(np.asarray(embeddings, dtype=np.float32))
    labels_np = np.asarray(labels)
    res = _run_device(attributes, embeddings, labels_np)
    return _combine(res.results, labels_np)


# revision 15
# speedup vs baseline: 1.1248x; 1.1248x over previous
"""HardTripletLoss Trainium2 kernel.

Reference computation (B=256, C=1000, D=300):
  relations[b,c] = ||emb[b*C+c] - att[b*C+c] + 1e-6||_2          [B, C]
  hardest_positive[c] = max_b relations[b,c] * onehot(labels)[b,c]
  mx[c]              = max_b relations[b,c]
  hardest_negative[c] = min_b (relations[b,c] + mx[c]*onehot[b,c])
  loss = sum(relu(hp - hn + 1)) / (count(relu(...) > 1e-16) + 1e-16)

Sharding: data-parallel over B across 8 cores (32 b's per core, each a
contiguous 32000-row chunk of the (B*C, D) tensors). Each core computes
squared distances rel_sq[P, R, BL] (class c = R*p + r) and 4 per-class
partial reductions [C] over local b:
  cmax = max_b rel_sq            pmax = max_b over positives of rel_sq
  umin = min_b over negatives    mmin = min_b over positives
(masking is exact: +/-1e30 select-style masks via min/max ALU ops, no
additive-cancellation error). Host all-reduces the [4, C] partials over
cores, takes sqrt (monotone, commutes with max/min), and finishes the
tiny [C]-sized tail: hn = min(umin, cmax_r + mmin), loss scalar.

DMA strategy (memory-bound: 76.8 MB/core of f32 input). Measured HW
facts driving the design:
  - the binding resource mid-stream is per-SDMA-engine throughput
    (~25.5 of ~29 GB/s read-side x 16 engines ~= 404 GB/s). The loads
    CAST f32->f16 in flight (SWDGE CCE) so the SBUF write side never
    binds. (f16 rounding is ~4e-6 relative on the loss; gate is 2e-2.)
  - all 64 chunk loads are spread over the 4 SWDGE rings (4 fixed DMA
    engines each) -- plain gpsimd dma_start is pinned to ring 0 by
    bass, so the InstDMACopy queue name is retargeted post-hoc to
    qPoolDynamic{1,2,3}. The 2 HWDGE queues share engines 0-4 and
    cannot cast; they carry only the tiny mask/out transfers.
  - partition p holds R=8 consecutive rows (c = 8p + r): each per-b load
    is a dense 1.2 MB 2D DMA, 9.6 KB contiguous HBM lines, 125 descs.
  - DMA-instruction issue on the in-order Pool engine costs ~0.7us
    each, so fewer+bigger DMAs win (half-splitting every chunk was a
    33% regression). The last chunk pair instead gets DEDICATED tiles:
    those 4 whole-chunk loads issue with no tile-free wait, killing an
    ~8us end-of-stream issue stall. io bufs=7 for the streamed chunks
    (6 and 8 are both measurably worse).
Compute: DVE subtracts e-a (f16); ACT Square+accum_out makes the row
sums for 6 of 8 rows (PSUM junk output keeps its writes off SBUF), DVE
mult+reduce the other 2 (4/4 near the end so tail buffers free sooner).
rel_sq is stored r-major [P, R, BL] so the masked per-class reductions
are 7 whole-tile DVE ops (a per-(r,b-range) formulation was 56 tiny ops
= ~12us of pure tail at ~205ns/op). One wave covers b<BL-NT mid-stream;
the last NT raw rel columns are copied (idle GpSimd) next to the wave
partials in one contiguous [P, 4+NT, R] tile and ship to the host in a
single dense output DMA; the host folds those NT b's into the
all-reduce in numpy -- so after the last DMA byte the device only runs
sub + squares + copy + a 16 KB store (~8us tail instead of ~22us).
"""

import numpy as np

B, C, D = 256, 1000, 300
M = 8            # cores
BL = B // M      # 32 local anchors per core
P = 125          # partitions; partition p holds classes c = R*p + r
R = C // P       # 8 consecutive rows per partition
NT = 5           # trailing b's whose raw rel columns go to the host
BIG = 1.0e30
EPS_PD = 1e-6
MARGIN = 1.0

_STATE = {}


def _build():
    import concourse.tile as tile
    from concourse import bacc, mybir

    nc = bacc.Bacc("TRN2", target_bir_lowering=False, debug=False,
                   num_devices=M, num_swdge_queues=4)
    dt = mybir.dt.float32
    emb = nc.dram_tensor("emb", [BL * C, D], dt, kind="ExternalInput").ap()
    att = nc.dram_tensor("att", [BL * C, D], dt, kind="ExternalInput").ap()
    msk = nc.dram_tensor("msk", [P, R, BL], dt, kind="ExternalInput").ap()
    out = nc.dram_tensor("out", [P, 4 + NT, R], dt, kind="ExternalOutput").ap()

    emb_v = emb.rearrange("(b p r) d -> b p r d", b=BL, p=P, r=R)
    att_v = att.rearrange("(b p r) d -> b p r d", b=BL, p=P, r=R)
    f16 = mybir.dt.float16

    Alu = mybir.AluOpType
    Act = mybir.ActivationFunctionType
    Ax = mybir.AxisListType

    def swdge(dst, src, q):
        """gpsimd dma_start retargeted to SWDGE ring q (0-3)."""
        inst = nc.gpsimd.dma_start(dst, src)
        if q:
            inst.ins.queue = f"qPoolDynamic{q}"
        return inst

    with tile.TileContext(nc) as tc:
        with (
            tc.tile_pool(name="io", bufs=7) as io_pool,
            tc.tile_pool(name="dif", bufs=2) as dif_pool,
            tc.tile_pool(name="small", bufs=1) as small_pool,
            tc.psum_pool(name="ps", bufs=2) as ps_pool,
        ):
            mask_t = small_pool.tile([P, R, BL], dt, tag="mask")
            nc.sync.dma_start(mask_t[:], msk[:])
            mask2_t = small_pool.tile([P, R, BL], dt, tag="mask2")
            nc.vector.tensor_scalar_mul(mask2_t[:], mask_t[:], -1.0)
            # rel_t[:, r, b] holds rel_sq of (b, c=R*p+r) -- r-major so the
            # reduce wave is whole-tile ops. Only the first BL-NT b's land
            # here; the NT tail chunks accumulate straight into part_t.
            rel_t = small_pool.tile([P, R, BL - NT], dt, tag="rel")
            # ACT's mandatory main output goes to PSUM: junk f32 writes to
            # SBUF (1.2 MB/chunk) were stealing SBUF write bandwidth from DMA
            junk_t = ps_pool.tile([P, D], dt, tag="junk", bufs=1)
            sq_t = ps_pool.tile([P, 4, D], dt, tag="sq", bufs=1)
            # three independent scratch buffers: with a single one the
            # tt->reduce->tt->reduce chain serializes on its WAR hazard and
            # the scheduler strings the wave out one op per chunk-gap,
            # finishing after the last chunk instead of before it
            w1_t = small_pool.tile([P, R, BL - NT], dt, tag="w1")
            w2_t = small_pool.tile([P, R, BL - NT], dt, tag="w2")
            w3_t = small_pool.tile([P, R, BL - NT], dt, tag="w3")
            # rows 0-3: wave partials; rows 4..4+NT: raw rel tail columns.
            # One contiguous tile -> ONE dense 144 B/partition output DMA (a
            # strided source was 1000x 12 B descriptors = ~4.5 us).
            part_t = small_pool.tile([P, 4 + NT, R], dt, tag="part")

            def reduce_wave(part, b0, b1):
                """Masked per-class partial min/max over b in [b0, b1).

                7 whole-tile DVE ops (a per-(r,b-range) formulation was 56
                tiny ops = ~12 us of pure tail at ~205ns/op). The scheduler
                statically interleaves them into the trailing chunks' DVE
                gaps; the 3 tt ops are mutually independent so they pack.
                Masking via min/max with +-1e30 select masks is exact.
                (GpSimd can't take them: its tensor_reduce is axis=C only.)
                """
                n = b1 - b0
                rel_v = rel_t[:, :, b0:b1]
                m_v = mask_t[:, :, b0:b1]
                m2_v = mask2_t[:, :, b0:b1]
                w1 = w1_t[:, :, :n]
                w2 = w2_t[:, :, :n]
                w3 = w3_t[:, :, :n]
                nc.vector.tensor_tensor(w1, rel_v, m_v, op=Alu.min)
                nc.vector.tensor_tensor(w2, rel_v, m_v, op=Alu.max)
                nc.vector.tensor_tensor(w3, rel_v, m2_v, op=Alu.max)
                nc.vector.tensor_reduce(part[:, 0, :], rel_v,
                                        axis=Ax.X, op=Alu.max)
                nc.vector.tensor_reduce(part[:, 1, :], w1,
                                        axis=Ax.X, op=Alu.max)
                nc.vector.tensor_reduce(part[:, 2, :], w2,
                                        axis=Ax.X, op=Alu.min)
                nc.vector.tensor_reduce(part[:, 3, :], w3,
                                        axis=Ax.X, op=Alu.min)

            # f32 -> f16 cast during the DMA (SWDGE CCE): halves the
            # SBUF-write bytes; the f16 rounding is far under the 2e-2 gate.
            for b in range(BL):
                # whole contiguous 1.2 MB b-chunk, 9.6 KB partition lines.
                # The last chunk pair gets dedicated buffers: those loads
                # issue with no tile-free wait (the in-order issue engine
                # stalled ~8us there otherwise), and the 4 full loads land
                # one per ring so the final drain runs ring-parallel.
                if b < BL - 2:
                    e_t = io_pool.tile([P, R, D], f16, tag="e")
                    a_t = io_pool.tile([P, R, D], f16, tag="a")
                else:
                    e_t = io_pool.tile([P, R, D], f16, tag=f"e_tl{b & 1}",
                                       bufs=1)
                    a_t = io_pool.tile([P, R, D], f16, tag=f"a_tl{b & 1}",
                                       bufs=1)
                swdge(e_t[:], emb_v[b], (2 * b) % 4)
                swdge(a_t[:], att_v[b], (2 * b + 1) % 4)
                d_t = dif_pool.tile([P, R, D], f16, tag="d")
                nc.vector.tensor_sub(d_t[:], e_t[:], a_t[:])
                # split the square+rowsum rows between ACT and DVE; even the
                # split (4/4) for the last 10 chunks so tail buffers free
                # sooner and the final loads issue without stalling
                n_dve = 2 if b < BL - 10 else 4
                # the NT tail chunks' row-sums accumulate STRAIGHT into the
                # output staging tile (the host finishes those b's): after
                # the last chunk's squares the out DMA fires immediately --
                # no copy step on the critical chain
                if b < BL - NT:
                    act_dst = lambda r: rel_t[:, r, b:b + 1]
                    dve_dst = rel_t[:, R - n_dve:, b:b + 1]
                else:
                    j = 4 + b - (BL - NT)
                    act_dst = lambda r: part_t[:, j, r:r + 1]
                    dve_dst = part_t[:, j, R - n_dve:]
                for r in range(R - n_dve):
                    # accum_out gives the 300-wide row-sum for free (f32)
                    nc.scalar.activation(
                        junk_t[:], d_t[:, r, :], Act.Square,
                        bias=0.0, scale=1.0,
                        accum_out=act_dst(r),
                    )
                nc.vector.tensor_tensor(
                    sq_t[:, :n_dve], d_t[:, R - n_dve:, :],
                    d_t[:, R - n_dve:, :], op=Alu.mult)
                nc.vector.tensor_reduce(
                    dve_dst, sq_t[:, :n_dve],
                    axis=Ax.X, op=Alu.add)
                if b == BL - NT - 1:
                    # one wave over b < BL-NT, overlapped with the last NT
                    # chunks' DMA + compute
                    reduce_wave(part_t, 0, BL - NT)

            nc.sync.dma_start(out[:], part_t[:])
    nc.compile()
    return nc


def _get_nc():
    if "nc" not in _STATE:
        _STATE["nc"] = _build()
    return _STATE["nc"]


def _make_masks(labels_np):
    """Per-core select masks msk[p, r, b] = +BIG if labels[b]==R*p+r else -BIG."""
    masks = []
    c_of_pr = R * np.arange(P)[:, None] + np.arange(R)[None, :]     # [P, R]
    for m in range(M):
        lb = labels_np[m * BL:(m + 1) * BL].astype(np.int64)        # [BL]
        match = c_of_pr[:, :, None] == lb[None, None, :]            # [P, R, BL]
        mask = np.where(match, np.float32(BIG), np.float32(-BIG))
        masks.append(np.ascontiguousarray(mask, dtype=np.float32))
    return masks


def _run_device(attributes, embeddings, labels_np, trace=False):
    from concourse.bass_utils import run_bass_kernel_spmd
    nc = _get_nc()
    masks = _make_masks(labels_np)
    in_maps = []
    for m in range(M):
        sl = slice(m * BL * C, (m + 1) * BL * C)
        in_maps.append({
            "emb": embeddings[sl],
            "att": attributes[sl],
            "msk": masks[m],
        })
    return run_bass_kernel_spmd(nc, in_maps, list(range(M)), trace=trace)


def _combine(results, labels_np):
    """All-reduce the per-core partials (plus NT raw tail columns) and
    finish the loss on host."""
    cmax = np.full(C, -np.inf)
    pmax = np.full(C, -np.inf)
    umin = np.full(C, np.inf)
    mmin = np.full(C, np.inf)
    for m in range(M):
        # device out [P, 4+NT, R] (class c = R*p + r) -> [4+NT, C]
        pk = np.transpose(results[m]["out"].astype(np.float64),
                          (1, 0, 2)).reshape(4 + NT, C)
        cmax = np.maximum(cmax, pk[0])
        pmax = np.maximum(pmax, pk[1])
        umin = np.minimum(umin, pk[2])
        mmin = np.minimum(mmin, pk[3])
        # raw rel_sq for the NT trailing b's
        tail = pk[4:]
        for j in range(NT):
            lb = int(labels_np[m * BL + BL - NT + j])
            row = tail[j]
            cmax = np.maximum(cmax, row)
            neg = row.copy()
            neg[lb] = np.inf
            umin = np.minimum(umin, neg)
            pmax[lb] = max(pmax[lb], row[lb])
            mmin[lb] = min(mmin[lb], row[lb])
    # squared space -> distances (max/min commute with sqrt on [0, inf))
    mx = np.sqrt(np.maximum(cmax, 0.0))
    hp = np.sqrt(np.clip(pmax, 0.0, None))    # -BIG (no positive) -> 0
    umin_r = np.sqrt(np.clip(umin, 0.0, None))  # +BIG sentinel stays huge
    mmin_r = np.sqrt(np.clip(mmin, 0.0, None))
    hn = np.minimum(umin_r, mx + mmin_r)
    triplet = np.maximum(hp - hn + MARGIN, 0.0)
    num_hard = np.sum(triplet > 1e-16)
    loss = np.sum(triplet) / (num_hard + 1e-16)
    return np.float32(loss)


def kernel(attributes, embeddings, labels):
    attributes = np.ascontiguousarray(np.asarray(attributes, dtype=np.float32))
    embeddings = np.ascontiguousarray(np.asarray(embeddings, dtype=np.float32))
    labels_np = np.asarray(labels)
    res = _run_device(attributes, embeddings, labels_np)
    return _combine(res.results, labels_np)


# revision 16
# speedup vs baseline: 1.1302x; 1.0048x over previous
"""HardTripletLoss Trainium2 kernel.

Reference computation (B=256, C=1000, D=300):
  relations[b,c] = ||emb[b*C+c] - att[b*C+c] + 1e-6||_2          [B, C]
  hardest_positive[c] = max_b relations[b,c] * onehot(labels)[b,c]
  mx[c]              = max_b relations[b,c]
  hardest_negative[c] = min_b (relations[b,c] + mx[c]*onehot[b,c])
  loss = sum(relu(hp - hn + 1)) / (count(relu(...) > 1e-16) + 1e-16)

Sharding: data-parallel over B across 8 cores (32 b's per core, each a
contiguous 32000-row chunk of the (B*C, D) tensors). Each core computes
squared distances rel_sq[P, R, BL] (class c = R*p + r) and 4 per-class
partial reductions [C] over local b:
  cmax = max_b rel_sq            pmax = max_b over positives of rel_sq
  umin = min_b over negatives    mmin = min_b over positives
(masking is exact: +/-1e30 select-style masks via min/max ALU ops, no
additive-cancellation error). Host all-reduces the [4, C] partials over
cores, takes sqrt (monotone, commutes with max/min), and finishes the
tiny [C]-sized tail: hn = min(umin, cmax_r + mmin), loss scalar.

DMA strategy (memory-bound: 76.8 MB/core of f32 input). Measured HW
facts driving the design:
  - the binding resource mid-stream is per-SDMA-engine throughput
    (~25.5 of ~29 GB/s read-side x 16 engines ~= 404 GB/s). The loads
    CAST f32->f16 in flight (SWDGE CCE) so the SBUF write side never
    binds. (f16 rounding is ~4e-6 relative on the loss; gate is 2e-2.)
  - all 64 chunk loads are spread over the 4 SWDGE rings (4 fixed DMA
    engines each) -- plain gpsimd dma_start is pinned to ring 0 by
    bass, so the InstDMACopy queue name is retargeted post-hoc to
    qPoolDynamic{1,2,3}. The 2 HWDGE queues share engines 0-4 and
    cannot cast; they carry only the tiny mask/out transfers.
  - partition p holds R=8 consecutive rows (c = 8p + r): each per-b load
    is a dense 1.2 MB 2D DMA, 9.6 KB contiguous HBM lines, 125 descs.
  - DMA-instruction issue on the in-order Pool engine costs ~0.7us
    each, so fewer+bigger DMAs win (half-splitting every chunk was a
    33% regression). The last chunk pair instead gets DEDICATED tiles:
    those 4 whole-chunk loads issue with no tile-free wait, killing an
    ~8us end-of-stream issue stall. io bufs=7 for the streamed chunks
    (6 and 8 are both measurably worse).
Compute: DVE subtracts e-a (f16); ACT Square+accum_out makes the row
sums for 6 of 8 rows (PSUM junk output keeps its writes off SBUF), DVE
mult+reduce the other 2 (4/4 near the end so tail buffers free sooner).
rel_sq is stored r-major [P, R, BL] so the masked per-class reductions
are 7 whole-tile DVE ops (a per-(r,b-range) formulation was 56 tiny ops
= ~12us of pure tail at ~205ns/op). One wave covers b<BL-NT mid-stream;
the last NT raw rel columns are copied (idle GpSimd) next to the wave
partials in one contiguous [P, 4+NT, R] tile and ship to the host in a
single dense output DMA; the host folds those NT b's into the
all-reduce in numpy -- so after the last DMA byte the device only runs
sub + squares + copy + a 16 KB store (~8us tail instead of ~22us).
"""

import numpy as np

B, C, D = 256, 1000, 300
M = 8            # cores
BL = B // M      # 32 local anchors per core
P = 125          # partitions; partition p holds classes c = R*p + r
R = C // P       # 8 consecutive rows per partition
NT = 5           # trailing b's whose raw rel columns go to the host
BIG = 1.0e30
EPS_PD = 1e-6
MARGIN = 1.0

_STATE = {}


def _build():
    import concourse.tile as tile
    from concourse import bacc, mybir

    nc = bacc.Bacc("TRN2", target_bir_lowering=False, debug=False,
                   num_devices=M, num_swdge_queues=4)
    dt = mybir.dt.float32
    emb = nc.dram_tensor("emb", [BL * C, D], dt, kind="ExternalInput").ap()
    att = nc.dram_tensor("att", [BL * C, D], dt, kind="ExternalInput").ap()
    msk = nc.dram_tensor("msk", [P, R, BL], dt, kind="ExternalInput").ap()
    out = nc.dram_tensor("out", [P, 4 + NT, R], dt, kind="ExternalOutput").ap()

    emb_v = emb.rearrange("(b p r) d -> b p r d", b=BL, p=P, r=R)
    att_v = att.rearrange("(b p r) d -> b p r d", b=BL, p=P, r=R)
    f16 = mybir.dt.float16

    Alu = mybir.AluOpType
    Act = mybir.ActivationFunctionType
    Ax = mybir.AxisListType

    def swdge(dst, src, q):
        """gpsimd dma_start retargeted to SWDGE ring q (0-3)."""
        inst = nc.gpsimd.dma_start(dst, src)
        if q:
            inst.ins.queue = f"qPoolDynamic{q}"
        return inst

    with tile.TileContext(nc) as tc:
        with (
            tc.tile_pool(name="io", bufs=7) as io_pool,
            tc.tile_pool(name="dif", bufs=2) as dif_pool,
            tc.tile_pool(name="small", bufs=1) as small_pool,
            tc.psum_pool(name="ps", bufs=2) as ps_pool,
        ):
            mask_t = small_pool.tile([P, R, BL], dt, tag="mask")
            nc.sync.dma_start(mask_t[:], msk[:])
            mask2_t = small_pool.tile([P, R, BL], dt, tag="mask2")
            nc.vector.tensor_scalar_mul(mask2_t[:], mask_t[:], -1.0)
            # rel_t[:, r, b] holds rel_sq of (b, c=R*p+r) -- r-major so the
            # reduce wave is whole-tile ops. Only the first BL-NT b's land
            # here; the NT tail chunks accumulate straight into part_t.
            rel_t = small_pool.tile([P, R, BL - NT], dt, tag="rel")
            # ACT's mandatory main output goes to PSUM: junk f32 writes to
            # SBUF (1.2 MB/chunk) were stealing SBUF write bandwidth from DMA
            junk_t = ps_pool.tile([P, D], dt, tag="junk", bufs=1)
            sq_t = ps_pool.tile([P, 4, D], dt, tag="sq", bufs=1)
            # three independent scratch buffers: with a single one the
            # tt->reduce->tt->reduce chain serializes on its WAR hazard and
            # the scheduler strings the wave out one op per chunk-gap,
            # finishing after the last chunk instead of before it
            w1_t = small_pool.tile([P, R, BL - NT], dt, tag="w1")
            w2_t = small_pool.tile([P, R, BL - NT], dt, tag="w2")
            w3_t = small_pool.tile([P, R, BL - NT], dt, tag="w3")
            # rows 0-3: wave partials; rows 4..4+NT: raw rel tail columns.
            # One contiguous tile -> ONE dense 144 B/partition output DMA (a
            # strided source was 1000x 12 B descriptors = ~4.5 us).
            part_t = small_pool.tile([P, 4 + NT, R], dt, tag="part")

            def reduce_wave(part, b0, b1):
                """Masked per-class partial min/max over b in [b0, b1).

                7 whole-tile DVE ops (a per-(r,b-range) formulation was 56
                tiny ops = ~12 us of pure tail at ~205ns/op). The scheduler
                statically interleaves them into the trailing chunks' DVE
                gaps; the 3 tt ops are mutually independent so they pack.
                Masking via min/max with +-1e30 select masks is exact.
                (GpSimd can't take them: its tensor_reduce is axis=C only.)
                """
                n = b1 - b0
                rel_v = rel_t[:, :, b0:b1]
                m_v = mask_t[:, :, b0:b1]
                m2_v = mask2_t[:, :, b0:b1]
                w1 = w1_t[:, :, :n]
                w2 = w2_t[:, :, :n]
                w3 = w3_t[:, :, :n]
                nc.vector.tensor_tensor(w1, rel_v, m_v, op=Alu.min)
                nc.vector.tensor_tensor(w2, rel_v, m_v, op=Alu.max)
                nc.vector.tensor_tensor(w3, rel_v, m2_v, op=Alu.max)
                nc.vector.tensor_reduce(part[:, 0, :], rel_v,
                                        axis=Ax.X, op=Alu.max)
                nc.vector.tensor_reduce(part[:, 1, :], w1,
                                        axis=Ax.X, op=Alu.max)
                nc.vector.tensor_reduce(part[:, 2, :], w2,
                                        axis=Ax.X, op=Alu.min)
                nc.vector.tensor_reduce(part[:, 3, :], w3,
                                        axis=Ax.X, op=Alu.min)

            # f32 -> f16 cast during the DMA (SWDGE CCE): halves the
            # SBUF-write bytes; the f16 rounding is far under the 2e-2 gate.
            H = R // 2

            def act_rows(d_t, rows, dst):
                for r in rows:
                    # accum_out gives the 300-wide row-sum for free (f32)
                    nc.scalar.activation(
                        junk_t[:], d_t[:, r, :], Act.Square,
                        bias=0.0, scale=1.0,
                        accum_out=dst(r),
                    )

            def dve_rows(d_t, r0, r1, dst, sq0=0):
                n = r1 - r0
                nc.vector.tensor_tensor(
                    sq_t[:, sq0:sq0 + n], d_t[:, r0:r1, :],
                    d_t[:, r0:r1, :], op=Alu.mult)
                nc.vector.tensor_reduce(dst, sq_t[:, sq0:sq0 + n],
                                        axis=Ax.X, op=Alu.add)

            for b in range(BL):
                # whole contiguous 1.2 MB b-chunk, 9.6 KB partition lines.
                # The last chunk pair gets dedicated buffers: those loads
                # issue with no tile-free wait (the in-order issue engine
                # stalled ~8us there otherwise).
                if b < BL - 2:
                    e_t = io_pool.tile([P, R, D], f16, tag="e")
                    a_t = io_pool.tile([P, R, D], f16, tag="a")
                else:
                    e_t = io_pool.tile([P, R, D], f16, tag=f"e_tl{b & 1}",
                                       bufs=1)
                    a_t = io_pool.tile([P, R, D], f16, tag=f"a_tl{b & 1}",
                                       bufs=1)
                qe, qa = (2 * b) % 4, (2 * b + 1) % 4
                d_t = dif_pool.tile([P, R, D], f16, tag="d")
                if b < BL - 2:
                    swdge(e_t[:], emb_v[b], qe)
                    swdge(a_t[:], att_v[b], qa)
                    nc.vector.tensor_sub(d_t[:], e_t[:], a_t[:])
                    # split the square+rowsum rows between ACT and DVE; even
                    # the split (4/4) for the late chunks so tail buffers
                    # free sooner and the final loads issue without stalling
                    n_dve = 2 if b < BL - 10 else 4
                    if b < BL - NT:
                        act_rows(d_t, range(R - n_dve),
                                 lambda r: rel_t[:, r, b:b + 1])
                        dve_rows(d_t, R - n_dve, R,
                                 rel_t[:, R - n_dve:, b:b + 1])
                    else:
                        # tail chunks accumulate STRAIGHT into the output
                        # staging tile (the host finishes those b's)
                        j = 4 + b - (BL - NT)
                        act_rows(d_t, range(R - n_dve),
                                 lambda r: part_t[:, j, r:r + 1])
                        dve_rows(d_t, R - n_dve, R, part_t[:, j, R - n_dve:])
                else:
                    # last two chunks: row-halved loads (halves land ~6 us
                    # apart on their ring) so most of the square/row-sum
                    # work runs while the second half still streams, and
                    # the post-last-byte chain is just sub-half + 2 ACT
                    # rows in parallel with 2 DVE rows + the 28 KB store
                    j = 4 + b - (BL - NT)
                    swdge(e_t[:, :H, :], emb_v[b][:, :H, :], qe)
                    swdge(a_t[:, :H, :], att_v[b][:, :H, :], qa)
                    swdge(e_t[:, H:, :], emb_v[b][:, H:, :], qe)
                    swdge(a_t[:, H:, :], att_v[b][:, H:, :], qa)
                    nc.vector.tensor_sub(d_t[:, :H, :], e_t[:, :H, :],
                                         a_t[:, :H, :])
                    if b == BL - 2:
                        act_rows(d_t, range(4),
                                 lambda r: part_t[:, j, r:r + 1])
                    else:
                        act_rows(d_t, range(3),
                                 lambda r: part_t[:, j, r:r + 1])
                        dve_rows(d_t, 3, 4, part_t[:, j, 3:4], sq0=2)
                    nc.vector.tensor_sub(d_t[:, H:, :], e_t[:, H:, :],
                                         a_t[:, H:, :])
                    act_rows(d_t, range(4, 6),
                             lambda r: part_t[:, j, r:r + 1])
                    dve_rows(d_t, 6, R, part_t[:, j, 6:])
                if b == BL - NT - 1:
                    # one wave over b < BL-NT, overlapped with the last NT
                    # chunks' DMA + compute
                    reduce_wave(part_t, 0, BL - NT)

            nc.sync.dma_start(out[:], part_t[:])
    nc.compile()
    return nc


def _get_nc():
    if "nc" not in _STATE:
        _STATE["nc"] = _build()
    return _STATE["nc"]


def _make_masks(labels_np):
    """Per-core select masks msk[p, r, b] = +BIG if labels[b]==R*p+r else -BIG."""
    masks = []
    c_of_pr = R * np.arange(P)[:, None] + np.arange(R)[None, :]     # [P, R]
    for m in range(M):
        lb = labels_np[m * BL:(m + 1) * BL].astype(np.int64)        # [BL]
        match = c_of_pr[:, :, None] == lb[None, None, :]            # [P, R, BL]
        mask = np.where(match, np.float32(BIG), np.float32(-BIG))
        masks.append(np.ascontiguousarray(mask, dtype=np.float32))
    return masks


def _run_device(attributes, embeddings, labels_np, trace=False):
    from concourse.bass_utils import run_bass_kernel_spmd
    nc = _get_nc()
    masks = _make_masks(labels_np)
    in_maps = []
    for m in range(M):
        sl = slice(m * BL * C, (m + 1) * BL * C)
        in_maps.append({
            "emb": embeddings[sl],
            "att": attributes[sl],
            "msk": masks[m],
        })
    return run_bass_kernel_spmd(nc, in_maps, list(range(M)), trace=trace)


def _combine(results, labels_np):
    """All-reduce the per-core partials (plus NT raw tail columns) and
    finish the loss on host."""
    cmax = np.full(C, -np.inf)
    pmax = np.full(C, -np.inf)
    umin = np.full(C, np.inf)
    mmin = np.full(C, np.inf)
    for m in range(M):
        # device out [P, 4+NT, R] (class c = R*p + r) -> [4+NT, C]
        pk = np.transpose(results[m]["out"].astype(np.float64),
                          (1, 0, 2)).reshape(4 + NT, C)
        cmax = np.maximum(cmax, pk[0])
        pmax = np.maximum(pmax, pk[1])
        umin = np.minimum(umin, pk[2])
        mmin = np.minimum(mmin, pk[3])
        # raw rel_sq for the NT trailing b's
        tail = pk[4:]
        for j in range(NT):
            lb = int(labels_np[m * BL + BL - NT + j])
            row = tail[j]
            cmax = np.maximum(cmax, row)
            neg = row.copy()
            neg[lb] = np.inf
            umin = np.minimum(umin, neg)
            pmax[lb] = max(pmax[lb], row[lb])
            mmin[lb] = min(mmin[lb], row[lb])
    # squared space -> distances (max/min commute with sqrt on [0, inf))
    mx = np.sqrt(np.maximum(cmax, 0.0))
    hp = np.sqrt(np.clip(pmax, 0.0, None))    # -BIG (no positive) -> 0
    umin_r = np.sqrt(np.clip(umin, 0.0, None))  # +BIG sentinel stays huge
    mmin_r = np.sqrt(np.clip(mmin, 0.0, None))
    hn = np.minimum(umin_r, mx + mmin_r)
    triplet = np.maximum(hp - hn + MARGIN, 0.0)
    num_hard = np.sum(triplet > 1e-16)
    loss = np.sum(triplet) / (num_hard + 1e-16)
    return np.float32(loss)


def kernel(attributes, embeddings, labels):
    attributes = np.ascontiguousarray(np.asarray(attributes, dtype=np.float32))
    embeddings = np.ascontiguousarray(np.asarray(embeddings, dtype=np.float32))
    labels_np = np.asarray(labels)
    res = _run_device(attributes, embeddings, labels_np)
    return _combine(res.results, labels_np)
